# revision 1
# baseline (speedup 1.0000x reference)
"""Multi-head causal attention with RoPE for TRN2, 8 NeuronCores.

Problem: B=2, T=2048, D=2048, 16 heads x head_dim 128, fp32.
  qkv = x @ Wqkv.T + bqkv ; RoPE(q, k) interleaved-pairs; causal softmax attention;
  out = attn_out @ Wo.T + bo.

Sharding: core c in 0..7 -> (batch b = c//4, head-group g = c%4 of 4 heads).
Each core computes its batch's partial output (its 4 heads' contribution through
the out-projection); host sums the 4 group partials per batch and adds bo.

Per-core kernel (all matmuls fp32r: full PE speed, ~1e-3 scale-relative rounding):
  Phase A: qkvT projection. q,k produced transposed [d, t] with head_dim rows
    de-interleaved (even dims then odd dims) so RoPE's rotate-half becomes a
    half-swap along partitions, done via a permutation matmul on PE. RoPE is
    applied during the PSUM drain. k lands directly in persistent SBUF tiles
    (no DRAM roundtrip); q and v roundtrip through DRAM.
  Phase B: per 512-wide q-tile, per head:
    S^T[k,q] = kT.T @ qT on PE, exp on ACT (scale 1/sqrt(dh) folded in),
    causal masks on diagonal blocks (DVE), PV accumulate O^T[d,q] (PE),
    denominator by ones-matmul (PE), reciprocal + partition-broadcast +
    normalize (DVE/GPSIMD).
  Phase C (interleaved per q-tile): final[t,o] += O^T_h[:,t].T @ WoT_h[:,o].
"""
import os
import sys

for _p in ("/opt/trn_rl_repo", "/root/.axon_site/_ro/trn_rl_repo"):
    if os.path.isdir(_p) and _p not in sys.path:
        sys.path.insert(0, _p)

import numpy as np

import concourse.bacc as bacc
import concourse.mybir as mybir
import concourse.tile as tile
from concourse.bass_utils import run_bass_kernel_spmd

dt = mybir.dt
AF = mybir.ActivationFunctionType

B = 2
T = 2048
D = 2048
NH = 16
HD = 128
ROPE_BASE = 10000.0
N_CORES = 8
GROUPS = 4          # head-groups (tensor-parallel axis)
HPG = NH // GROUPS  # heads per group = 4
FQK = HPG * HD      # 512: q (or k) feature cols per core
FV = HPG * HD       # 512
QT = 512            # q-tile width in attention
NQT = T // QT       # 4
NKC = T // 128      # 16 k-chunks
NCC = D // 128      # 16 contraction chunks
TB = 512            # phase-A t-block
NTB = T // TB       # 4
SCALE = 1.0 / float(np.sqrt(HD))


def build(loop=1):
    """Emit the per-core BIR program (identical for all 8 cores)."""
    import contextlib

    nc = bacc.Bacc("TRN2", target_bir_lowering=False, debug=False)

    xT_d = nc.dram_tensor("xT", [D, T], dt.float32r, kind="ExternalInput")
    wqp_d = nc.dram_tensor("wqpack", [8, 128, NCC * 128], dt.float32r,
                           kind="ExternalInput")
    wvp_d = nc.dram_tensor("wvpack", [128, NCC * FV], dt.float32r,
                           kind="ExternalInput")
    woT_d = nc.dram_tensor("woT", [FV, D], dt.float32r, kind="ExternalInput")
    cos_d = nc.dram_tensor("cosT", [HD, T], dt.float16, kind="ExternalInput")
    sin_d = nc.dram_tensor("sinT", [HD, T], dt.float16, kind="ExternalInput")
    mask_d = nc.dram_tensor("masks", [4, HD, QT], dt.float32, kind="ExternalInput")
    bqk_d = nc.dram_tensor("bqk", [2 * FQK, 1], dt.float32, kind="ExternalInput")
    bv_d = nc.dram_tensor("bvb", [HD, FV], dt.float32, kind="ExternalInput")
    ones_d = nc.dram_tensor("ones", [HD, 1], dt.float32r, kind="ExternalInput")
    perm_d = nc.dram_tensor("perm", [HD, HD], dt.float32r, kind="ExternalInput")
    out_d = nc.dram_tensor("outp", [T, D], dt.float32, kind="ExternalOutput")

    with tile.TileContext(nc, pool_alloc_mode="queue") as tc:
        with (
            tc.For_i(0, loop, 1) if loop > 1 else contextlib.nullcontext(),
            tc.tile_pool(name="dram", bufs=1, space="DRAM") as dramp,
            tc.tile_pool(name="kres", bufs=1) as kres,
        ):
            qT_tbs, vN_tbs = [], []
            for tb in range(NTB):
                qT_tb = dramp.tile([FQK, TB], dt.float32r, tag=f"qT{tb}",
                                   name=f"qT_{tb}")
                qT_tbs.append(qT_tb)
                vN_tb = dramp.tile([TB, FV], dt.float32r, tag=f"vN{tb}",
                                   name=f"vN_{tb}")
                vN_tbs.append(vN_tb)

            k_rs = []
            for h in range(HPG):
                k_r = kres.tile([HD, T], dt.float32r, tag=f"kr{h}", name=f"kr_{h}")
                k_rs.append(k_r)

            # -------- Phase A: qkv projection + RoPE on q,k (during drain) --------
            with (
                tc.tile_pool(name="wq", bufs=1) as wpool,
                tc.tile_pool(name="xb", bufs=2) as xpool,
                tc.tile_pool(name="adr", bufs=2) as adrain,
                tc.tile_pool(name="arope", bufs=2) as arope,
                tc.tile_pool(name="abias", bufs=1) as abias,
                tc.tile_pool(name="aps", bufs=2, space="PSUM") as aps,
                tc.tile_pool(name="rps", bufs=2, space="PSUM") as rps,
            ):
                bqk_sb = abias.tile([128, 8, 1], dt.float32)
                nc.scalar.dma_start(
                    out=bqk_sb, in_=bqk_d.ap().rearrange("(f p) o -> p f o", p=128)
                )
                bv_sb = abias.tile([HD, FV], dt.float32)
                nc.scalar.dma_start(out=bv_sb, in_=bv_d.ap())
                cos_t = abias.tile([HD, T], dt.float16)
                sin_t = abias.tile([HD, T], dt.float16)
                nc.scalar.dma_start(out=cos_t, in_=cos_d.ap())
                nc.scalar.dma_start(out=sin_t, in_=sin_d.ap())
                perm_t = abias.tile([HD, HD], dt.float32r)
                nc.scalar.dma_start(out=perm_t, in_=perm_d.ap())

                def load_xb(tb):
                    tsl = slice(tb * TB, (tb + 1) * TB)
                    xbl = []
                    for cc in range(NCC):
                        xb_c = xpool.tile(
                            [128, TB], dt.float32r, tag=f"xb{cc}",
                            name=f"xb_{tb}_{cc}", bufs=(1 if cc >= 14 else 2),
                        )
                        nc.sync.dma_start(
                            out=xb_c,
                            in_=xT_d.ap()[cc * 128:(cc + 1) * 128, tsl],
                        )
                        xbl.append(xb_c)
                    return xbl

                # first t-block's activations win the sync queue
                xb0 = load_xb(0)

                # qk weights as 8 column-blocks, split across both HWDGE
                # queues (evens+v on scalar, odds on sync behind xb0)
                wq_blocks = [None] * 8
                for fb, eng in ((0, nc.scalar), (4, nc.scalar), (1, nc.sync),
                                (5, nc.sync), (2, nc.scalar), (6, nc.scalar),
                                (3, nc.sync), (7, nc.sync)):
                    wq_b = wpool.tile([128, NCC, 128], dt.float32r, tag=f"wq{fb}",
                                      name=f"wq_{fb}")
                    eng.dma_start(
                        out=wq_b,
                        in_=wqp_d.ap()[fb].rearrange("p (cc f) -> p cc f", f=128),
                    )
                    wq_blocks[fb] = wq_b
                wv_b = wpool.tile([128, NCC, FV], dt.float32r)
                nc.scalar.dma_start(
                    out=wv_b,
                    in_=wvp_d.ap().rearrange("p (cc f) -> p cc f", f=FV),
                )
                for tb in range(NTB):
                    tsl = slice(tb * TB, (tb + 1) * TB)
                    xb = xb0 if tb == 0 else load_xb(tb)
                    # q,k: transposed [f, t]; RoPE during drain; k -> SBUF resident
                    for f in (0, 4, 1, 5, 2, 6, 3, 7):
                        ps = aps.tile([128, TB], dt.float32)
                        for cc in range(NCC):
                            nc.tensor.matmul(
                                ps,
                                wq_blocks[f][:, cc, :],
                                xb[cc],
                                start=(cc == 0),
                                stop=(cc == NCC - 1),
                            )
                        s1 = arope.tile([128, TB], dt.float32r, tag="s1")
                        nc.vector.tensor_scalar_add(s1, ps, bqk_sb[:, f, :])
                        rot_ps = rps.tile([128, TB], dt.float32)
                        nc.tensor.matmul(rot_ps, perm_t, s1, start=True, stop=True)
                        nc.vector.tensor_mul(out=s1, in0=s1, in1=cos_t[:, tsl])
                        nc.vector.tensor_mul(out=rot_ps, in0=rot_ps, in1=sin_t[:, tsl])
                        if f < 4:  # q -> DRAM roundtrip
                            dr = adrain.tile([128, TB], dt.float32r, tag="adr")
                            nc.vector.tensor_add(out=dr, in0=s1, in1=rot_ps)
                            nc.sync.dma_start(
                                out=qT_tbs[tb][f * 128:(f + 1) * 128, :], in_=dr,
                            )
                        else:      # k -> persistent SBUF
                            nc.vector.tensor_add(
                                out=k_rs[f - 4][:, tsl], in0=s1, in1=rot_ps
                            )
                    # v: natural output [t, d]
                    for ts4 in range(TB // 128):
                        ps = aps.tile([128, FV], dt.float32)
                        for cc in range(NCC):
                            nc.tensor.matmul(
                                ps,
                                xb[cc][:, ts4 * 128:(ts4 + 1) * 128],
                                wv_b[:, cc, :],
                                start=(cc == 0),
                                stop=(cc == NCC - 1),
                            )
                        dr = adrain.tile([128, FV], dt.float32r, tag="adr")
                        nc.vector.tensor_add(dr, ps, bv_sb)
                        nc.sync.dma_start(
                            out=vN_tbs[tb][ts4 * 128:(ts4 + 1) * 128, :],
                            in_=dr,
                        )

            # -------- Phase B + C: attention, out-proj per q-tile --------
            with (
                tc.tile_pool(name="bsing", bufs=1) as bsing,
                tc.tile_pool(name="qt", bufs=2) as qtp,
                tc.tile_pool(name="vt", bufs=1) as vtp,
                tc.tile_pool(name="osb", bufs=2) as osbp,
                tc.tile_pool(name="pt", bufs=4) as ptp,
                tc.tile_pool(name="bsmall", bufs=2) as bsmall,
                tc.tile_pool(name="wo", bufs=1) as wop,
                tc.tile_pool(name="cdr", bufs=3) as cdrain,
                tc.tile_pool(name="ps_s", bufs=3, space="PSUM") as ps_s,
                tc.tile_pool(name="ps_o", bufs=2, space="PSUM") as ps_o,
                tc.tile_pool(name="ps_l", bufs=1, space="PSUM") as ps_l,
                tc.tile_pool(name="cps", bufs=2, space="PSUM") as cps,
            ):
                mask_t = bsing.tile([HD, 4, QT], dt.float32)
                nc.sync.dma_start(out=mask_t, in_=mask_d.ap().transpose([1, 0, 2]))
                ones_t = bsing.tile([HD, 1], dt.float32r)
                nc.scalar.dma_start(out=ones_t, in_=ones_d.ap())

                # first q-tile's q loads win the queue; v chunks tb-major,
                # alternating the two HWDGE queues
                q_t0s = []
                for h in range(HPG):
                    q_t = qtp.tile([HD, QT], dt.float32r, tag=f"qt{h}",
                                   name=f"qt_0_{h}")
                    nc.scalar.dma_start(out=q_t, in_=qT_tbs[0][h * HD:(h + 1) * HD, :])
                    q_t0s.append(q_t)
                v_ts = []
                for h in range(HPG):
                    v_t = vtp.tile([128, NKC, HD], dt.float32r, tag=f"v{h}",
                                   name=f"v_{h}")
                    v_ts.append(v_t)
                qi = 0
                for tb in range(NTB):
                    for h in range(HPG):
                        eng = nc.scalar if (qi % 2 == 0) else nc.sync
                        qi += 1
                        eng.dma_start(
                            out=v_ts[h][:, 4 * tb:4 * (tb + 1), :],
                            in_=vN_tbs[tb][:, h * HD:(h + 1) * HD].rearrange(
                                "(c p) d -> p c d", p=128
                            ),
                        )

                wo_sb = wop.tile([128, HPG, D], dt.float32r)
                nc.sync.dma_start(
                    out=wo_sb, in_=woT_d.ap().rearrange("(hh p) o -> p hh o", p=128)
                )

                def emit_cproj(pj, o_hs, tts, on_act=False):
                    # out-projection tiles (tt in tts) for q-tile pj
                    for tt in tts:
                        for oo in range(D // QT):
                            ps = cps.tile([128, QT], dt.float32,
                                          name=f"cps_{pj}_{tt}_{oo}", tag="cps")
                            for h in range(HPG):
                                nc.tensor.matmul(
                                    ps,
                                    o_hs[h][:, tt * 128:(tt + 1) * 128],
                                    wo_sb[:, h, oo * QT:(oo + 1) * QT],
                                    start=(h == 0), stop=(h == HPG - 1),
                                )
                            dr = cdrain.tile([128, QT], dt.float32,
                                             name=f"cdr_{pj}_{tt}_{oo}", tag="cdr")
                            if on_act:
                                nc.scalar.copy(out=dr, in_=ps)
                            else:
                                nc.vector.tensor_copy(out=dr, in_=ps)
                            nc.sync.dma_start(
                                out=out_d.ap()[
                                    pj * QT + tt * 128: pj * QT + (tt + 1) * 128,
                                    oo * QT:(oo + 1) * QT,
                                ],
                                in_=dr,
                            )

                prev_o = None
                for j in range(NQT):
                    nkc = 4 * (j + 1)
                    o_heads = []
                    for h in range(HPG):
                        if j == 0:
                            q_t = q_t0s[h]
                        else:
                            q_t = qtp.tile([HD, QT], dt.float32r, tag=f"qt{h}",
                                           name=f"qt_{j}_{h}")
                            nc.scalar.dma_start(
                                out=q_t, in_=qT_tbs[j][h * HD:(h + 1) * HD, :]
                            )
                        o_head_tile = osbp.tile([HD, QT], dt.float32r, tag=f"osb{h}",
                                                name=f"osb_{j}_{h}")
                        o_heads.append(o_head_tile)
                        psum_o = ps_o.tile([HD, QT], dt.float32)
                        psum_l = ps_l.tile([1, QT], dt.float32)

                        def col0(kc):
                            m = kc - 4 * j
                            if m <= 0:
                                return 0
                            return 128 if m == 1 else 256

                        def s_matmul(kc):
                            c0 = col0(kc)
                            psum_s = ps_s.tile(
                                [128, QT], dt.float32,
                                name=f"s_{j}_{h}_{kc}", tag="psum_s",
                            )
                            nc.tensor.matmul(
                                psum_s[:, c0:],
                                k_rs[h][:, kc * 128:(kc + 1) * 128],
                                q_t[:, c0:],
                                start=True, stop=True,
                            )
                            return psum_s

                        s_next = s_matmul(0)
                        for kc in range(nkc):
                            psum_s = s_next
                            if kc + 1 < nkc:
                                s_next = s_matmul(kc + 1)
                            c0 = col0(kc)
                            pt = ptp.tile([128, QT], dt.float32r)
                            nc.scalar.activation(
                                out=pt[:, c0:], in_=psum_s[:, c0:],
                                func=AF.Exp, scale=SCALE,
                            )
                            m = kc - 4 * j
                            if m >= 0:
                                nc.vector.tensor_mul(
                                    out=pt[:, c0:], in0=pt[:, c0:],
                                    in1=mask_t[:, m, c0:],
                                )
                            nc.tensor.matmul(
                                psum_o[:, c0:], v_ts[h][:, kc, :], pt[:, c0:],
                                start=(kc == 0), stop=(kc == nkc - 1),
                            )
                            nc.tensor.matmul(
                                psum_l[:, c0:], ones_t, pt[:, c0:],
                                start=(kc == 0), stop=(kc == nkc - 1),
                            )
                        recip = bsmall.tile([1, QT], dt.float32, tag="recip")
                        nc.vector.reciprocal(out=recip, in_=psum_l)
                        bcast = bsmall.tile([128, QT], dt.float32, tag="bcast")
                        nc.gpsimd.partition_broadcast(bcast, recip)
                        nc.vector.tensor_mul(
                            out=o_heads[h], in0=psum_o, in1=bcast
                        )
                        # interleave previous q-tile's out-projection
                        if prev_o is not None:
                            emit_cproj(j - 1, prev_o, [h])
                    prev_o = o_heads
                emit_cproj(NQT - 1, prev_o, list(range(QT // 128)), on_act=True)
    nc.compile()
    return nc


# ---------------------------------------------------------------------------
# Host side
# ---------------------------------------------------------------------------

_DEINT = np.concatenate([np.arange(0, HD, 2), np.arange(1, HD, 2)])  # de-interleave


def _rope_tables():
    half = HD // 2
    inv_freq = 1.0 / (ROPE_BASE ** (np.arange(half, dtype=np.float64) / half))
    t = np.arange(T, dtype=np.float64)
    fr = t[None, :] * inv_freq[:, None]          # (64, T)
    cos = np.concatenate([np.cos(fr), np.cos(fr)], axis=0).astype(np.float16)
    sin = np.concatenate([-np.sin(fr), np.sin(fr)], axis=0).astype(np.float16)
    return cos, sin


def _masks():
    m = np.zeros((4, HD, QT), dtype=np.float32)
    kk = np.arange(HD)[:, None]
    qq = np.arange(QT)[None, :]
    for i in range(4):
        m[i] = (kk <= qq - 128 * i).astype(np.float32)
    return m


def _perm():
    p = np.zeros((HD, HD), dtype=np.float32)
    half = HD // 2
    for i in range(half):
        p[i + half, i] = 1.0   # rot[m<64]  = s1[m+64]
        p[i, i + half] = 1.0   # rot[m>=64] = s1[m-64]
    return p


def make_in_maps(x, Wqkv, bqkv, Wo, bo):
    cos, sin = _rope_tables()
    masks = _masks()
    ones = np.ones((HD, 1), dtype=np.float32)
    perm = _perm()

    Wq = Wqkv[0 * D:1 * D]
    Wk = Wqkv[1 * D:2 * D]
    Wv = Wqkv[2 * D:3 * D]
    bq = bqkv[0 * D:1 * D]
    bk = bqkv[1 * D:2 * D]
    bv = bqkv[2 * D:3 * D]

    in_maps = []
    for c in range(N_CORES):
        b, g = divmod(c, GROUPS)
        hsl = slice(g * HPG * HD, (g + 1) * HPG * HD)
        # de-interleaved row order for q,k heads of this group
        rows = np.arange(g * HPG * HD, (g + 1) * HPG * HD).reshape(HPG, HD)
        rows = rows[:, _DEINT].reshape(-1)

        wq = Wq[rows]                       # (512, D)
        wk = Wk[rows]
        wv = Wv[hsl]                        # natural order
        wqkT = np.concatenate([wq, wk], axis=0).T.astype(np.float32)  # (D, 1024)
        # packed [fb, p, cc*f]: per-partition contiguous DMA rows
        wqpack = np.ascontiguousarray(
            wqkT.reshape(NCC, 128, 8, 128)      # (cc, p, fb, f)
                .transpose(2, 1, 0, 3)           # (fb, p, cc, f)
                .reshape(8, 128, NCC * 128)
        )
        wvT = wv.T.astype(np.float32)            # (D, 512)
        wvpack = np.ascontiguousarray(
            wvT.reshape(NCC, 128, FV).transpose(1, 0, 2).reshape(128, NCC * FV)
        )
        woT = np.ascontiguousarray(Wo[:, hsl].T.astype(np.float32))  # (512, D)

        bqk = np.concatenate([bq[rows], bk[rows]]).astype(np.float32)[:, None]
        bvb = np.broadcast_to(bv[hsl].astype(np.float32), (HD, FV)).copy()

        xT = np.ascontiguousarray(np.asarray(x[b]).T.astype(np.float32))  # (D, T)

        in_maps.append({
            "xT": xT,
            "wqpack": wqpack,
            "wvpack": wvpack,
            "woT": woT,
            "cosT": cos,
            "sinT": sin,
            "masks": masks,
            "bqk": bqk,
            "bvb": bvb,
            "ones": ones,
            "perm": perm,
        })
    return in_maps


_NC_CACHE = {}


def _get_nc(loop=1):
    if loop not in _NC_CACHE:
        _NC_CACHE[loop] = build(loop=loop)
    return _NC_CACHE[loop]


def kernel(x, Wqkv, bqkv, Wo, bo):
    x = np.asarray(x)
    Wqkv = np.asarray(Wqkv)
    bqkv = np.asarray(bqkv)
    Wo = np.asarray(Wo)
    bo = np.asarray(bo)

    nc = _get_nc()
    in_maps = make_in_maps(x, Wqkv, bqkv, Wo, bo)
    res = run_bass_kernel_spmd(nc, in_maps, core_ids=list(range(N_CORES)))

    out = np.zeros((B, T, D), dtype=np.float32)
    for c in range(N_CORES):
        b = c // GROUPS
        out[b] += res.results[c]["outp"]
    out += bo.astype(np.float32)[None, None, :]
    return out



# revision 12
# speedup vs baseline: 1.0802x; 1.0802x over previous
"""Multi-head causal attention with RoPE for TRN2, 8 NeuronCores.

Problem: B=2, T=2048, D=2048, 16 heads x head_dim 128, fp32 reference.
  qkv = x @ Wqkv.T + bqkv ; RoPE(q, k) interleaved-pairs; causal softmax
  attention; out = attn_out @ Wo.T + bo.

Sharding: core c in 0..7 -> (batch b = c//4, head-group g = c%4 of 4 heads).
Each core computes its batch's partial output (its 4 heads' contribution
through the out-projection); host sums the 4 group partials per batch and
adds bo.

All matmul operands are bf16 (fp32 PSUM accumulation) - measured end-to-end
rel err ~4e-3 vs the fp32 reference, under the 2e-2 gate. bf16 keeps
q/k/v resident in SBUF (no DRAM roundtrips) and halves DMA + DVE work.
Inputs arrive via a small number of large packed DMAs (per-DMA queue +
semaphore overhead is ~1.5us, so small transfers are consolidated).

Per-core kernel:
  Phase A: qkvT projection. q,k produced transposed [d, t] with head_dim
    rows de-interleaved (even dims then odd dims) so RoPE's rotate-half is
    a half-swap along partitions, done via a permutation matmul on PE.
    Drain: ACT adds bias (psum->SBUF bf16), DVE applies cos/sin. q,k land
    in persistent SBUF tiles [128, T]; v drains to natural-layout SBUF
    tiles [128t, 512d] (PV stationary slices directly). The perm matmul
    for feature-block f is emitted after f+1's accumulation matmuls so PE
    never waits on ACT.
  Phase B: per 512-wide q-tile j, per head h: S^T[k,q] chunks on PE with
    exact causal trim (col offset 128*m on diagonal chunks), exp on ACT
    (scale 1/sqrt(dh) folded), narrow [128,128] triangle mask on DVE for
    diagonal chunks, PV accumulate O^T[d,q] + ones-matmul denominator on
    PE, reciprocal (DVE) + partition-broadcast (GPSIMD) + normalize (DVE).
  Phase C (interleaved per head): final[t,o] += O^T_h[:,t].T @ WoT_h[:,o];
    drains alternate DVE/ACT into a [128, 2048] bf16 row tile, one output
    DMA per 128 token rows.
"""
import os
import sys

for _p in ("/opt/trn_rl_repo", "/root/.axon_site/_ro/trn_rl_repo"):
    if os.path.isdir(_p) and _p not in sys.path:
        sys.path.insert(0, _p)

import numpy as np

import concourse.bacc as bacc
import concourse.mybir as mybir
import concourse.tile as tile
from concourse.bass_utils import run_bass_kernel_spmd

dt = mybir.dt
AF = mybir.ActivationFunctionType

B = 2
T = 2048
D = 2048
NH = 16
HD = 128
ROPE_BASE = 10000.0
N_CORES = 8
GROUPS = 4          # head-groups (tensor-parallel axis)
HPG = NH // GROUPS  # heads per group = 4
FQK = HPG * HD      # 512: q (or k) feature cols per core
FV = HPG * HD       # 512
QT = 512            # q-tile width in attention
NQT = T // QT       # 4
NKC = T // 128      # 16 k-chunks
NCC = D // 128      # 16 contraction chunks
TB = 512            # phase-A t-block
NTB = T // TB       # 4
SCALE = 1.0 / float(np.sqrt(HD))

F_ORDER = (0, 4, 1, 5, 2, 6, 3, 7)  # q/k feature-block consumption order


def build(loop=1):
    """Emit the per-core BIR program (identical for all 8 cores)."""
    import contextlib

    nc = bacc.Bacc("TRN2", target_bir_lowering=False, debug=False)

    xT_d = nc.dram_tensor("xT", [D, T], dt.bfloat16, kind="ExternalInput")
    # wqpack is pre-ordered on the host in F_ORDER consumption order
    wqp_d = nc.dram_tensor("wqpack", [8, 128, NCC * 128], dt.bfloat16,
                           kind="ExternalInput")
    wvp_d = nc.dram_tensor("wvpack", [128, NCC * FV], dt.bfloat16,
                           kind="ExternalInput")
    woT_d = nc.dram_tensor("woT", [FV, D], dt.bfloat16, kind="ExternalInput")
    css_d = nc.dram_tensor("css", [2, HD, T], dt.float16, kind="ExternalInput")
    # cst8: [:, :128]=perm, [:, 128:256]=tri, [:, 256]=ones
    cst8_d = nc.dram_tensor("cst8", [HD, 257], dt.bfloat16, kind="ExternalInput")
    # cst32: [:, :8]=bqk (per f-block, F_ORDER), [:, 8:]=bv broadcast
    cst32_d = nc.dram_tensor("cst32", [HD, 8 + FV], dt.float32,
                             kind="ExternalInput")
    out_d = nc.dram_tensor("outp", [T, D], dt.bfloat16, kind="ExternalOutput")

    with tile.TileContext(nc, pool_alloc_mode="queue") as tc:
        with (
            tc.For_i(0, loop, 1) if loop > 1 else contextlib.nullcontext(),
            tc.tile_pool(name="kres", bufs=1) as kres,
        ):
            # persistent SBUF: roped q,k [d, t] per head; v natural [t, d]
            k_rs, q_rs = [], []
            for h in range(HPG):
                k_rs.append(kres.tile([HD, T], dt.bfloat16, tag=f"kr{h}",
                                      name=f"kr_{h}"))
            for h in range(HPG):
                q_rs.append(kres.tile([HD, T], dt.bfloat16, tag=f"qr{h}",
                                      name=f"qr_{h}"))
            v_sb = []
            for kc in range(NKC):
                v_sb.append(kres.tile([128, FV], dt.bfloat16, tag=f"v{kc}",
                                      name=f"v_{kc}"))

            # -------- Phase A: qkv projection + RoPE on q,k --------
            with (
                tc.tile_pool(name="wq", bufs=1) as wpool,
                tc.tile_pool(name="xb", bufs=1) as xpool,
                tc.tile_pool(name="arope", bufs=2) as arope,
                tc.tile_pool(name="abias", bufs=1) as abias,
                tc.tile_pool(name="aps", bufs=2, space="PSUM") as aps,
                tc.tile_pool(name="rps", bufs=2, space="PSUM") as rps,
            ):
                # ---- DMA schedule: few large transfers, in consumption
                # order.  scalar queue: wq0 | cst | x0-hi | wq4 | wq(1,5) |
                # wq(2,6,3,7) | x2 | x3.  sync queue: x0-lo | css | x1 |
                # wv | wo.
                wq_sb = [None] * 8  # indexed by position in F_ORDER

                def load_wq(pos_lo, pos_hi, eng):
                    wq_b = wpool.tile([128, pos_hi - pos_lo, NCC, 128],
                                      dt.bfloat16, tag=f"wq{pos_lo}",
                                      name=f"wq_{pos_lo}")
                    eng.dma_start(
                        out=wq_b,
                        in_=wqp_d.ap()[pos_lo:pos_hi].rearrange(
                            "g p (cc f) -> p g cc f", f=128),
                    )
                    for i in range(pos_lo, pos_hi):
                        wq_sb[i] = wq_b[:, i - pos_lo]

                def load_xtb(tb, split=1):
                    xt = xpool.tile([128, NCC, TB], dt.bfloat16, tag="x",
                                    name=f"x_{tb}", bufs=2)
                    tsl = slice(tb * TB, (tb + 1) * TB)
                    step = NCC // split
                    for s in range(split):
                        csl = slice(s * step, (s + 1) * step)
                        eng = nc.sync if (s % 2 == 0) else nc.scalar
                        eng.dma_start(
                            out=xt[:, csl, :],
                            in_=xT_d.ap().rearrange(
                                "(cc p) t -> p cc t", p=128)[:, csl, tsl],
                        )
                    return xt

                # scalar: wq0 | x0[4:8] | x0[12:16] | cst32 | cst8 | wq(f1) |
                #         wq(f5) | wv
                # sync:   x0[0:4] | x0[8:12] | wq(f4) | css | wq(f2,f6,f3,f7)
                #         | x1 (| x2 | x3 | wo from the main loops)
                load_wq(0, 1, nc.scalar)
                x_tiles = [None] * NTB
                x_tiles[0] = load_xtb(0, split=4)
                load_wq(1, 2, nc.sync)
                cst32 = abias.tile([HD, 8 + FV], dt.float32)
                nc.scalar.dma_start(out=cst32, in_=cst32_d.ap())
                cst8 = kres.tile([HD, 257], dt.bfloat16, tag="cst8")
                nc.scalar.dma_start(out=cst8, in_=cst8_d.ap())
                load_wq(2, 3, nc.scalar)
                css = abias.tile([HD, 2, T], dt.float16)
                nc.sync.dma_start(
                    out=css, in_=css_d.ap().rearrange("s p t -> p s t"))
                load_wq(3, 4, nc.scalar)
                load_wq(4, 8, nc.sync)
                wv_b = wpool.tile([128, NCC, FV], dt.bfloat16)
                nc.scalar.dma_start(
                    out=wv_b,
                    in_=wvp_d.ap().rearrange("p (cc f) -> p cc f", f=FV),
                )
                x_tiles[1] = load_xtb(1)

                perm_t = cst8[:, 0:128]
                tri_t = cst8[:, 128:256]
                ones_t = cst8[:, 256:257]
                bqk_sb = cst32[:, 0:8]
                bv_sb = cst32[:, 8:]
                cos_t = css[:, 0]
                sin_t = css[:, 1]

                def rope_drain(fi, tb, s1):
                    """perm matmul + cos/sin combine for feature block f."""
                    f = F_ORDER[fi]
                    tsl = slice(tb * TB, (tb + 1) * TB)
                    rot = rps.tile([128, TB], dt.float32, tag="rot")
                    nc.tensor.matmul(rot, perm_t, s1, start=True, stop=True)
                    dst = (q_rs[f] if f < 4 else k_rs[f - 4])
                    m1 = arope.tile([128, TB], dt.bfloat16, tag="m1")
                    nc.vector.tensor_mul(out=m1, in0=s1, in1=cos_t[:, tsl])
                    m2 = arope.tile([128, TB], dt.bfloat16, tag="m2")
                    nc.vector.tensor_mul(out=m2, in0=rot, in1=sin_t[:, tsl])
                    nc.vector.tensor_add(out=dst[:, tsl], in0=m1, in1=m2)

                pending = None  # delayed perm-matmul drain (fi, tb, s1)
                for tb in range(NTB):
                    xt = x_tiles[tb] if x_tiles[tb] is not None else load_xtb(tb)
                    if tb + 1 < NTB and x_tiles[tb + 1] is None:
                        x_tiles[tb + 1] = load_xtb(tb + 1)
                    for fi in range(8):
                        ps = aps.tile([128, TB], dt.float32, tag="aps")
                        for cc in range(NCC):
                            nc.tensor.matmul(
                                ps,
                                wq_sb[fi][:, cc, :],
                                xt[:, cc, :],
                                start=(cc == 0),
                                stop=(cc == NCC - 1),
                            )
                        # bias add on ACT (psum -> SBUF bf16)
                        s1 = arope.tile([128, TB], dt.bfloat16, tag="s1")
                        nc.scalar.activation(
                            out=s1, in_=ps, func=AF.Identity,
                            bias=bqk_sb[:, fi:fi + 1],
                        )
                        if pending is not None:
                            rope_drain(*pending)
                        pending = (fi, tb, s1)
                    # v: natural output [t, d]
                    for ts4 in range(TB // 128):
                        kc = tb * 4 + ts4
                        ps = aps.tile([128, FV], dt.float32, tag="aps")
                        for cc in range(NCC):
                            nc.tensor.matmul(
                                ps,
                                xt[:, cc, ts4 * 128:(ts4 + 1) * 128],
                                wv_b[:, cc, :],
                                start=(cc == 0),
                                stop=(cc == NCC - 1),
                            )
                        if pending is not None:
                            rope_drain(*pending)
                            pending = None
                        nc.vector.tensor_add(out=v_sb[kc], in0=ps, in1=bv_sb)

            # -------- Phase B + C: attention, out-proj per q-tile --------
            with (
                tc.tile_pool(name="osb", bufs=2) as osbp,
                tc.tile_pool(name="pt", bufs=4) as ptp,
                tc.tile_pool(name="bsmall", bufs=2) as bsmall,
                tc.tile_pool(name="wo", bufs=1) as wop,
                tc.tile_pool(name="cdr", bufs=2) as cdrain,
                tc.tile_pool(name="ps_s", bufs=3, space="PSUM") as ps_s,
                tc.tile_pool(name="ps_o", bufs=2, space="PSUM") as ps_o,
                tc.tile_pool(name="ps_l", bufs=1, space="PSUM") as ps_l,
                tc.tile_pool(name="cps", bufs=2, space="PSUM") as cps,
            ):
                wo_sb = wop.tile([128, HPG, D], dt.bfloat16)
                nc.sync.dma_start(
                    out=wo_sb, in_=woT_d.ap().rearrange("(hh p) o -> p hh o", p=128)
                )

                def emit_cproj(pj, o_hs, tts, final=False):
                    # out-projection tiles (tt in tts) for q-tile pj;
                    # one [128, D] bf16 row drain + a single DMA per tt
                    # (per-oo DMAs on the final tile to shorten the tail)
                    for tt in tts:
                        dr = cdrain.tile([128, D], dt.bfloat16,
                                         name=f"cdr_{pj}_{tt}", tag="cdr")
                        rsl = slice(pj * QT + tt * 128, pj * QT + (tt + 1) * 128)
                        for oo in range(D // QT):
                            ps = cps.tile([128, QT], dt.float32,
                                          name=f"cps_{pj}_{tt}_{oo}", tag="cps")
                            for h in range(HPG):
                                nc.tensor.matmul(
                                    ps,
                                    o_hs[h][:, tt * 128:(tt + 1) * 128],
                                    wo_sb[:, h, oo * QT:(oo + 1) * QT],
                                    start=(h == 0), stop=(h == HPG - 1),
                                )
                            osl = slice(oo * QT, (oo + 1) * QT)
                            if oo % 2 == 0:
                                nc.scalar.copy(out=dr[:, osl], in_=ps)
                            else:
                                nc.vector.tensor_copy(out=dr[:, osl], in_=ps)
                            if final:
                                eng = nc.sync if oo % 2 == 0 else nc.scalar
                                eng.dma_start(
                                    out=out_d.ap()[rsl, osl], in_=dr[:, osl]
                                )
                        if not final:
                            nc.sync.dma_start(
                                out=out_d.ap()[rsl, :], in_=dr,
                            )

                prev_o = None
                for j in range(NQT):
                    nkc = 4 * (j + 1)
                    o_heads = []
                    for h in range(HPG):
                        q_t = q_rs[h]
                        o_head_tile = osbp.tile([HD, QT], dt.bfloat16,
                                                tag=f"osb{h}",
                                                name=f"osb_{j}_{h}")
                        o_heads.append(o_head_tile)
                        psum_o = ps_o.tile([HD, QT], dt.float32)
                        psum_l = ps_l.tile([1, QT], dt.float32)

                        def col0(kc):
                            m = kc - 4 * j
                            return 0 if m <= 0 else 128 * m

                        def s_matmul(kc):
                            c0 = col0(kc)
                            psum_s = ps_s.tile(
                                [128, QT], dt.float32,
                                name=f"s_{j}_{h}_{kc}", tag="psum_s",
                            )
                            nc.tensor.matmul(
                                psum_s[:, c0:],
                                k_rs[h][:, kc * 128:(kc + 1) * 128],
                                q_t[:, j * QT + c0:(j + 1) * QT],
                                start=True, stop=True,
                            )
                            return psum_s

                        # prefetch depth 2: PE never waits on ACT exp
                        s_q = [s_matmul(0), s_matmul(1)]
                        for kc in range(nkc):
                            psum_s = s_q.pop(0)
                            if kc + 2 < nkc:
                                s_q.append(s_matmul(kc + 2))
                            c0 = col0(kc)
                            m = kc - 4 * j
                            pt = ptp.tile([128, QT], dt.bfloat16)
                            nc.scalar.activation(
                                out=pt[:, c0:], in_=psum_s[:, c0:],
                                func=AF.Exp, scale=SCALE,
                            )
                            if m >= 0:
                                nc.vector.tensor_mul(
                                    out=pt[:, c0:c0 + 128],
                                    in0=pt[:, c0:c0 + 128],
                                    in1=tri_t,
                                )
                            nc.tensor.matmul(
                                psum_o[:, c0:], v_sb[kc][:, h * HD:(h + 1) * HD],
                                pt[:, c0:],
                                start=(kc == 0), stop=(kc == nkc - 1),
                            )
                            nc.tensor.matmul(
                                psum_l[:, c0:], ones_t, pt[:, c0:],
                                start=(kc == 0), stop=(kc == nkc - 1),
                            )
                        recip = bsmall.tile([1, QT], dt.float32, tag="recip")
                        nc.vector.reciprocal(out=recip, in_=psum_l)
                        bcast = bsmall.tile([128, QT], dt.float32, tag="bcast")
                        nc.gpsimd.partition_broadcast(bcast, recip)
                        nc.vector.tensor_mul(
                            out=o_heads[h], in0=psum_o, in1=bcast
                        )
                        # interleave previous q-tile's out-projection
                        if prev_o is not None:
                            emit_cproj(j - 1, prev_o, [h])
                    prev_o = o_heads
                emit_cproj(NQT - 1, prev_o, list(range(QT // 128 - 1)))
                emit_cproj(NQT - 1, prev_o, [QT // 128 - 1], final=True)
    nc.compile()
    return nc


# ---------------------------------------------------------------------------
# Host side
# ---------------------------------------------------------------------------

_DEINT = np.concatenate([np.arange(0, HD, 2), np.arange(1, HD, 2)])  # de-interleave


def _bf16(a):
    import ml_dtypes
    return np.ascontiguousarray(np.asarray(a).astype(ml_dtypes.bfloat16))


def _rope_tables():
    half = HD // 2
    inv_freq = 1.0 / (ROPE_BASE ** (np.arange(half, dtype=np.float64) / half))
    t = np.arange(T, dtype=np.float64)
    fr = t[None, :] * inv_freq[:, None]          # (64, T)
    cos = np.concatenate([np.cos(fr), np.cos(fr)], axis=0)
    sin = np.concatenate([-np.sin(fr), np.sin(fr)], axis=0)
    return np.stack([cos, sin]).astype(np.float16)  # (2, HD, T)


def _cst8():
    # [HD, 257]: perm | tri | ones
    p = np.zeros((HD, HD), dtype=np.float32)
    half = HD // 2
    for i in range(half):
        p[i + half, i] = 1.0   # rot[m<64]  = s1[m+64]
        p[i, i + half] = 1.0   # rot[m>=64] = s1[m-64]
    kk = np.arange(HD)[:, None]
    qq = np.arange(128)[None, :]
    tri = (kk <= qq).astype(np.float32)
    ones = np.ones((HD, 1), dtype=np.float32)
    return _bf16(np.concatenate([p, tri, ones], axis=1))


def make_in_maps(x, Wqkv, bqkv, Wo, bo):
    css = _rope_tables()
    cst8 = _cst8()

    Wq = Wqkv[0 * D:1 * D]
    Wk = Wqkv[1 * D:2 * D]
    Wv = Wqkv[2 * D:3 * D]
    bq = bqkv[0 * D:1 * D]
    bk = bqkv[1 * D:2 * D]
    bv = bqkv[2 * D:3 * D]

    in_maps = []
    for c in range(N_CORES):
        b, g = divmod(c, GROUPS)
        hsl = slice(g * HPG * HD, (g + 1) * HPG * HD)
        # de-interleaved row order for q,k heads of this group
        rows = np.arange(g * HPG * HD, (g + 1) * HPG * HD).reshape(HPG, HD)
        rows = rows[:, _DEINT].reshape(-1)

        wq = Wq[rows]                       # (512, D)
        wk = Wk[rows]
        wv = Wv[hsl]                        # natural order
        wqkT = np.concatenate([wq, wk], axis=0).T.astype(np.float32)  # (D, 1024)
        # packed [fb, p, cc*f] in F_ORDER consumption order
        wqpack = wqkT.reshape(NCC, 128, 8, 128)  # (cc, p, fb, f)
        wqpack = wqpack[:, :, F_ORDER, :]
        wqpack = _bf16(
            wqpack.transpose(2, 1, 0, 3).reshape(8, 128, NCC * 128)
        )
        wvT = wv.T.astype(np.float32)            # (D, 512)
        wvpack = _bf16(
            wvT.reshape(NCC, 128, FV).transpose(1, 0, 2).reshape(128, NCC * FV)
        )
        woT = _bf16(Wo[:, hsl].T.astype(np.float32))  # (512, D)

        bqk = np.concatenate([bq[rows], bk[rows]]).astype(np.float32)
        bqk = bqk.reshape(8, 128)[list(F_ORDER)].T  # (128, 8) F_ORDER cols
        bvb = np.broadcast_to(bv[hsl].astype(np.float32), (HD, FV))
        cst32 = np.ascontiguousarray(
            np.concatenate([bqk, bvb], axis=1).astype(np.float32)
        )

        xT = _bf16(np.asarray(x[b]).T)  # (D, T)

        in_maps.append({
            "xT": xT,
            "wqpack": wqpack,
            "wvpack": wvpack,
            "woT": woT,
            "css": css,
            "cst8": cst8,
            "cst32": cst32,
        })
    return in_maps


_NC_CACHE = {}


def _get_nc(loop=1):
    if loop not in _NC_CACHE:
        _NC_CACHE[loop] = build(loop=loop)
    return _NC_CACHE[loop]


def kernel(x, Wqkv, bqkv, Wo, bo):
    x = np.asarray(x)
    Wqkv = np.asarray(Wqkv)
    bqkv = np.asarray(bqkv)
    Wo = np.asarray(Wo)
    bo = np.asarray(bo)

    nc = _get_nc()
    in_maps = make_in_maps(x, Wqkv, bqkv, Wo, bo)
    res = run_bass_kernel_spmd(nc, in_maps, core_ids=list(range(N_CORES)))

    out = np.zeros((B, T, D), dtype=np.float32)
    for c in range(N_CORES):
        b = c // GROUPS
        out[b] += res.results[c]["outp"].astype(np.float32)
    out += bo.astype(np.float32)[None, None, :]
    return out


# revision 17
# speedup vs baseline: 1.5285x; 1.4149x over previous
"""Multi-head causal attention with RoPE for TRN2, 8 NeuronCores.

Problem: B=2, T=2048, D=2048, 16 heads x head_dim 128, fp32 reference.
  qkv = x @ Wqkv.T + bqkv ; RoPE(q, k) interleaved-pairs; causal softmax
  attention; out = attn_out @ Wo.T + bo.

Sharding: core c in 0..7 -> (batch b = c//4, head-group g = c%4 of 4 heads).
Each core computes its batch's partial output (its 4 heads' contribution
through the out-projection); host sums the 4 group partials per batch and
adds bo.

All matmul operands are bf16 (fp32 PSUM accumulation) - measured end-to-end
rel err ~4e-3 vs the fp32 reference, under the 2e-2 gate. bf16 keeps
q/k/v resident in SBUF (no DRAM roundtrips) and halves DMA + DVE work.
Inputs arrive via a small number of large packed DMAs (per-DMA queue +
semaphore overhead is ~1.5us, so small transfers are consolidated).

Per-core kernel:
  Phase A: qkvT projection. q,k produced transposed [d, t] with head_dim
    rows de-interleaved (even dims then odd dims) so RoPE's rotate-half is
    a half-swap along partitions, done via a permutation matmul on PE.
    Drain: ACT adds bias (psum->SBUF bf16), DVE applies cos/sin. q,k land
    in persistent SBUF tiles [128, T]; v drains to natural-layout SBUF
    tiles [128t, 512d] (PV stationary slices directly). The perm matmul
    for feature-block f is emitted after f+1's accumulation matmuls so PE
    never waits on ACT.
  Phase B: per 512-wide q-tile j, per head h: S^T[k,q] chunks on PE with
    exact causal trim (col offset 128*m on diagonal chunks), exp on ACT
    (scale 1/sqrt(dh) folded), narrow [128,128] triangle mask on DVE for
    diagonal chunks, PV accumulate O^T[d,q] + ones-matmul denominator on
    PE, reciprocal (DVE) + partition-broadcast (GPSIMD) + normalize (DVE).
  Phase C (interleaved per head): final[t,o] += O^T_h[:,t].T @ WoT_h[:,o];
    drains alternate DVE/ACT into a [128, 2048] bf16 row tile, one output
    DMA per 128 token rows.
"""
import os
import sys

for _p in ("/opt/trn_rl_repo", "/root/.axon_site/_ro/trn_rl_repo"):
    if os.path.isdir(_p) and _p not in sys.path:
        sys.path.insert(0, _p)

import numpy as np

import concourse.bacc as bacc
import concourse.mybir as mybir
import concourse.tile as tile
from concourse.bass_utils import run_bass_kernel_spmd

dt = mybir.dt
AF = mybir.ActivationFunctionType

B = 2
T = 2048
D = 2048
NH = 16
HD = 128
ROPE_BASE = 10000.0
N_CORES = 8
GROUPS = 4          # head-groups (tensor-parallel axis)
HPG = NH // GROUPS  # heads per group = 4
FQK = HPG * HD      # 512: q (or k) feature cols per core
FV = HPG * HD       # 512
QT = 512            # q-tile width in attention
NQT = T // QT       # 4
NKC = T // 128      # 16 k-chunks
NCC = D // 128      # 16 contraction chunks
TB = 512            # phase-A t-block
NTB = T // TB       # 4
SCALE = 1.0 / float(np.sqrt(HD))

F_ORDER = (0, 4, 1, 5, 2, 6, 3, 7)  # q/k feature-block consumption order


def build(loop=1):
    """Emit the per-core BIR program (identical for all 8 cores)."""
    import contextlib

    nc = bacc.Bacc("TRN2", target_bir_lowering=False, debug=False)

    xT_d = nc.dram_tensor("xT", [D, T], dt.bfloat16, kind="ExternalInput")
    # wqpack is pre-ordered on the host in F_ORDER consumption order
    wqp_d = nc.dram_tensor("wqpack", [8, 128, NCC * 128], dt.bfloat16,
                           kind="ExternalInput")
    wvp_d = nc.dram_tensor("wvpack", [128, NCC * FV], dt.bfloat16,
                           kind="ExternalInput")
    woT_d = nc.dram_tensor("woT", [FV, D], dt.bfloat16, kind="ExternalInput")
    css_d = nc.dram_tensor("css", [2, HD, T], dt.float16, kind="ExternalInput")
    # cst8: [:, :128]=perm, [:, 128:256]=tri, [:, 256]=ones
    cst8_d = nc.dram_tensor("cst8", [HD, 257], dt.bfloat16, kind="ExternalInput")
    # cst32: [:, :8]=bqk (per f-block, F_ORDER), [:, 8:]=bv broadcast
    cst32_d = nc.dram_tensor("cst32", [HD, 8 + FV], dt.float32,
                             kind="ExternalInput")
    out_d = nc.dram_tensor("outp", [T, D], dt.bfloat16, kind="ExternalOutput")

    with tile.TileContext(nc, pool_alloc_mode="queue") as tc:
        with (
            tc.For_i(0, loop, 1) if loop > 1 else contextlib.nullcontext(),
            tc.tile_pool(name="kres", bufs=1) as kres,
        ):
            # persistent SBUF: roped q,k [d, t] per head; v natural [t, d]
            k_rs, q_rs = [], []
            for h in range(HPG):
                k_rs.append(kres.tile([HD, T], dt.bfloat16, tag=f"kr{h}",
                                      name=f"kr_{h}"))
            for h in range(HPG):
                q_rs.append(kres.tile([HD, T], dt.bfloat16, tag=f"qr{h}",
                                      name=f"qr_{h}"))
            v_sb = []
            for kc in range(NKC):
                v_sb.append(kres.tile([128, FV], dt.bfloat16, tag=f"v{kc}",
                                      name=f"v_{kc}"))

            # -------- Phase A: qkv projection + RoPE on q,k --------
            with (
                tc.tile_pool(name="wq", bufs=1) as wpool,
                tc.tile_pool(name="xb", bufs=1) as xpool,
                tc.tile_pool(name="arope", bufs=2) as arope,
                tc.tile_pool(name="abias", bufs=1) as abias,
                tc.tile_pool(name="aps", bufs=2, space="PSUM") as aps,
                tc.tile_pool(name="rps", bufs=2, space="PSUM") as rps,
            ):
                # ---- DMA schedule: few large transfers, in consumption
                # order.  scalar queue: wq0 | cst | x0-hi | wq4 | wq(1,5) |
                # wq(2,6,3,7) | x2 | x3.  sync queue: x0-lo | css | x1 |
                # wv | wo.
                wq_sb = [None] * 8  # indexed by position in F_ORDER

                def load_wq(pos_lo, pos_hi, eng):
                    wq_b = wpool.tile([128, pos_hi - pos_lo, NCC, 128],
                                      dt.bfloat16, tag=f"wq{pos_lo}",
                                      name=f"wq_{pos_lo}")
                    eng.dma_start(
                        out=wq_b,
                        in_=wqp_d.ap()[pos_lo:pos_hi].rearrange(
                            "g p (cc f) -> p g cc f", f=128),
                    )
                    for i in range(pos_lo, pos_hi):
                        wq_sb[i] = wq_b[:, i - pos_lo]

                def load_xtb(tb, split=1):
                    xt = xpool.tile([128, NCC, TB], dt.bfloat16, tag="x",
                                    name=f"x_{tb}", bufs=2)
                    tsl = slice(tb * TB, (tb + 1) * TB)
                    step = NCC // split
                    for s in range(split):
                        csl = slice(s * step, (s + 1) * step)
                        eng = nc.sync if (s % 2 == 0) else nc.scalar
                        eng.dma_start(
                            out=xt[:, csl, :],
                            in_=xT_d.ap().rearrange(
                                "(cc p) t -> p cc t", p=128)[:, csl, tsl],
                        )
                    return xt

                # scalar: wq0 | x0[4:8] | x0[12:16] | cst32 | cst8 | wq(f1) |
                #         wq(f5) | wv
                # sync:   x0[0:4] | x0[8:12] | wq(f4) | css | wq(f2,f6,f3,f7)
                #         | x1 (| x2 | x3 | wo from the main loops)
                load_wq(0, 1, nc.scalar)
                x_tiles = [None] * NTB
                x_tiles[0] = load_xtb(0, split=4)
                load_wq(1, 2, nc.sync)
                cst32 = abias.tile([HD, 8 + FV], dt.float32)
                nc.scalar.dma_start(out=cst32, in_=cst32_d.ap())
                cst8 = kres.tile([HD, 257], dt.bfloat16, tag="cst8")
                nc.scalar.dma_start(out=cst8, in_=cst8_d.ap())
                load_wq(2, 3, nc.scalar)
                css = abias.tile([HD, 2, T], dt.float16)
                nc.sync.dma_start(
                    out=css, in_=css_d.ap().rearrange("s p t -> p s t"))
                load_wq(3, 4, nc.scalar)
                load_wq(4, 8, nc.sync)
                wv_b = wpool.tile([128, NCC, FV], dt.bfloat16)
                nc.scalar.dma_start(
                    out=wv_b,
                    in_=wvp_d.ap().rearrange("p (cc f) -> p cc f", f=FV),
                )
                x_tiles[1] = load_xtb(1)

                perm_t = cst8[:, 0:128]
                tri_t = cst8[:, 128:256]
                ones_t = cst8[:, 256:257]
                bqk_sb = cst32[:, 0:8]
                bv_sb = cst32[:, 8:]
                cos_t = css[:, 0]
                sin_t = css[:, 1]

                def rope_drain(fi, tb, s1):
                    """perm matmul + cos/sin combine for feature block f."""
                    f = F_ORDER[fi]
                    tsl = slice(tb * TB, (tb + 1) * TB)
                    rot = rps.tile([128, TB], dt.float32, tag="rot")
                    nc.tensor.matmul(rot, perm_t, s1, start=True, stop=True)
                    dst = (q_rs[f] if f < 4 else k_rs[f - 4])
                    m1 = arope.tile([128, TB], dt.bfloat16, tag="m1")
                    nc.vector.tensor_mul(out=m1, in0=s1, in1=cos_t[:, tsl])
                    m2 = arope.tile([128, TB], dt.bfloat16, tag="m2")
                    nc.vector.tensor_mul(out=m2, in0=rot, in1=sin_t[:, tsl])
                    nc.vector.tensor_add(out=dst[:, tsl], in0=m1, in1=m2)

                pending = None  # delayed perm-matmul drain (fi, tb, s1)
                for tb in range(NTB):
                    xt = x_tiles[tb] if x_tiles[tb] is not None else load_xtb(tb)
                    if tb + 1 < NTB and x_tiles[tb + 1] is None:
                        x_tiles[tb + 1] = load_xtb(tb + 1)
                    for fi in range(8):
                        ps = aps.tile([128, TB], dt.float32, tag="aps")
                        for cc in range(NCC):
                            nc.tensor.matmul(
                                ps,
                                wq_sb[fi][:, cc, :],
                                xt[:, cc, :],
                                start=(cc == 0),
                                stop=(cc == NCC - 1),
                            )
                        # bias add on ACT (psum -> SBUF bf16)
                        s1 = arope.tile([128, TB], dt.bfloat16, tag="s1")
                        nc.scalar.activation(
                            out=s1, in_=ps, func=AF.Identity,
                            bias=bqk_sb[:, fi:fi + 1],
                        )
                        if pending is not None:
                            rope_drain(*pending)
                        pending = (fi, tb, s1)
                    # v: natural output [t, d]
                    for ts4 in range(TB // 128):
                        kc = tb * 4 + ts4
                        ps = aps.tile([128, FV], dt.float32, tag="aps")
                        for cc in range(NCC):
                            nc.tensor.matmul(
                                ps,
                                xt[:, cc, ts4 * 128:(ts4 + 1) * 128],
                                wv_b[:, cc, :],
                                start=(cc == 0),
                                stop=(cc == NCC - 1),
                            )
                        if pending is not None:
                            rope_drain(*pending)
                            pending = None
                        nc.vector.tensor_add(out=v_sb[kc], in0=ps, in1=bv_sb)

            # -------- Phase B + C: attention, out-proj per q-tile --------
            with (
                tc.tile_pool(name="osb", bufs=2) as osbp,
                tc.tile_pool(name="pt", bufs=4) as ptp,
                tc.tile_pool(name="bsmall", bufs=2) as bsmall,
                tc.tile_pool(name="wo", bufs=1) as wop,
                tc.tile_pool(name="cdr", bufs=3) as cdrain,
                tc.tile_pool(name="ps_s", bufs=2, space="PSUM") as ps_s,
                tc.tile_pool(name="ps_o", bufs=1, space="PSUM") as ps_o,
                tc.tile_pool(name="ps_l", bufs=1, space="PSUM") as ps_l,
                tc.tile_pool(name="cps", bufs=2, space="PSUM") as cps,
            ):
                wo_sb = wop.tile([128, HPG, D], dt.bfloat16)
                nc.sync.dma_start(
                    out=wo_sb, in_=woT_d.ap().rearrange("(hh p) o -> p hh o", p=128)
                )

                def emit_cproj(pj, o_hs, tts, final=False):
                    # out-projection tiles (tt in tts) for q-tile pj;
                    # one [128, D] bf16 row drain + a single DMA per tt
                    # (per-oo DMAs on the final tile to shorten the tail)
                    for tt in tts:
                        dr = cdrain.tile([128, D], dt.bfloat16,
                                         name=f"cdr_{pj}_{tt}", tag="cdr")
                        rsl = slice(pj * QT + tt * 128, pj * QT + (tt + 1) * 128)
                        for oo in range(D // QT):
                            ps = cps.tile([128, QT], dt.float32,
                                          name=f"cps_{pj}_{tt}_{oo}", tag="cps")
                            for h in range(HPG):
                                nc.tensor.matmul(
                                    ps,
                                    o_hs[h][:, tt * 128:(tt + 1) * 128],
                                    wo_sb[:, h, oo * QT:(oo + 1) * QT],
                                    start=(h == 0), stop=(h == HPG - 1),
                                )
                            osl = slice(oo * QT, (oo + 1) * QT)
                            if oo == 0:
                                nc.scalar.copy(out=dr[:, osl], in_=ps)
                            else:
                                nc.vector.tensor_copy(out=dr[:, osl], in_=ps)
                            if final:
                                eng = nc.sync if oo % 2 == 0 else nc.scalar
                                eng.dma_start(
                                    out=out_d.ap()[rsl, osl], in_=dr[:, osl]
                                )
                        if not final:
                            nc.sync.dma_start(
                                out=out_d.ap()[rsl, :], in_=dr,
                            )

                prev_o = None
                for j in range(NQT):
                    nkc = 4 * (j + 1)
                    o_heads = []
                    for h in range(HPG):
                        q_t = q_rs[h]
                        o_head_tile = osbp.tile([HD, QT], dt.bfloat16,
                                                tag=f"osb{h}",
                                                name=f"osb_{j}_{h}")
                        o_heads.append(o_head_tile)
                        psum_o = ps_o.tile([HD, QT], dt.float32)
                        psum_l = ps_l.tile([1, QT], dt.float32)

                        def col0(kc):
                            m = kc - 4 * j
                            return 0 if m <= 0 else 128 * m

                        def s_pair(p):
                            # S^T matmuls for chunk pair (2p, 2p+1) into one
                            # 2-bank psum tile, then exp: one ACT instruction
                            # for off-diagonal pairs, per-chunk on diagonal
                            # pairs (column offsets differ)
                            psum_s = ps_s.tile(
                                [128, 2, QT], dt.float32,
                                name=f"s_{j}_{h}_{p}", tag="psum_s",
                            )
                            for i in (0, 1):
                                kc = 2 * p + i
                                c0 = col0(kc)
                                nc.tensor.matmul(
                                    psum_s[:, i, c0:],
                                    k_rs[h][:, kc * 128:(kc + 1) * 128],
                                    q_t[:, j * QT + c0:(j + 1) * QT],
                                    start=True, stop=True,
                                )
                            pt = ptp.tile([128, 2, QT], dt.bfloat16,
                                          name=f"pt_{j}_{h}_{p}", tag="pt")
                            if col0(2 * p + 1) == 0:
                                nc.scalar.activation(
                                    out=pt, in_=psum_s, func=AF.Exp, scale=SCALE,
                                )
                            else:
                                for i in (0, 1):
                                    c0 = col0(2 * p + i)
                                    nc.scalar.activation(
                                        out=pt[:, i, c0:], in_=psum_s[:, i, c0:],
                                        func=AF.Exp, scale=SCALE,
                                    )
                            return pt

                        # Diagonal pairs first (their 4 serial per-chunk exps
                        # are hidden behind the interleaved cproj matmuls),
                        # then off-diagonal pairs with 1-pair prefetch.
                        # PSUM accumulation order is irrelevant; start= goes
                        # on chunk 4j (col0==0, full width), stop= on the
                        # last-emitted chunk.
                        npair = nkc // 2
                        pair_order = [npair - 2, npair - 1] + list(range(npair - 2))
                        start_kc = 4 * j
                        stop_kc = 2 * pair_order[-1] + 1

                        def consume(pt, p):
                            for i in (0, 1):
                                kc = 2 * p + i
                                c0 = col0(kc)
                                m = kc - 4 * j
                                if m >= 0:
                                    nc.vector.tensor_mul(
                                        out=pt[:, i, c0:c0 + 128],
                                        in0=pt[:, i, c0:c0 + 128],
                                        in1=tri_t,
                                    )
                                nc.tensor.matmul(
                                    psum_o[:, c0:],
                                    v_sb[kc][:, h * HD:(h + 1) * HD],
                                    pt[:, i, c0:],
                                    start=(kc == start_kc), stop=(kc == stop_kc),
                                )
                                nc.tensor.matmul(
                                    psum_l[:, c0:], ones_t, pt[:, i, c0:],
                                    start=(kc == start_kc), stop=(kc == stop_kc),
                                )

                        p_q = [s_pair(pair_order[0]), s_pair(pair_order[1])]
                        # previous q-tile's out-projection lands here: its
                        # matmuls cover the diagonal exps' latency
                        if prev_o is not None:
                            emit_cproj(j - 1, prev_o, [h])
                        for idx, p in enumerate(pair_order):
                            pt = p_q.pop(0)
                            if idx + 2 < npair:
                                p_q.append(s_pair(pair_order[idx + 2]))
                            consume(pt, p)
                        recip = bsmall.tile([1, QT], dt.float32, tag="recip")
                        nc.vector.reciprocal(out=recip, in_=psum_l)
                        bcast = bsmall.tile([128, QT], dt.float32, tag="bcast")
                        nc.gpsimd.partition_broadcast(bcast, recip)
                        nc.vector.tensor_mul(
                            out=o_heads[h], in0=psum_o, in1=bcast
                        )
                    prev_o = o_heads
                emit_cproj(NQT - 1, prev_o, list(range(QT // 128 - 1)))
                emit_cproj(NQT - 1, prev_o, [QT // 128 - 1], final=True)
    nc.compile()
    return nc


# ---------------------------------------------------------------------------
# Host side
# ---------------------------------------------------------------------------

_DEINT = np.concatenate([np.arange(0, HD, 2), np.arange(1, HD, 2)])  # de-interleave


def _bf16(a):
    import ml_dtypes
    return np.ascontiguousarray(np.asarray(a).astype(ml_dtypes.bfloat16))


def _rope_tables():
    half = HD // 2
    inv_freq = 1.0 / (ROPE_BASE ** (np.arange(half, dtype=np.float64) / half))
    t = np.arange(T, dtype=np.float64)
    fr = t[None, :] * inv_freq[:, None]          # (64, T)
    cos = np.concatenate([np.cos(fr), np.cos(fr)], axis=0)
    sin = np.concatenate([-np.sin(fr), np.sin(fr)], axis=0)
    return np.stack([cos, sin]).astype(np.float16)  # (2, HD, T)


def _cst8():
    # [HD, 257]: perm | tri | ones
    p = np.zeros((HD, HD), dtype=np.float32)
    half = HD // 2
    for i in range(half):
        p[i + half, i] = 1.0   # rot[m<64]  = s1[m+64]
        p[i, i + half] = 1.0   # rot[m>=64] = s1[m-64]
    kk = np.arange(HD)[:, None]
    qq = np.arange(128)[None, :]
    tri = (kk <= qq).astype(np.float32)
    ones = np.ones((HD, 1), dtype=np.float32)
    return _bf16(np.concatenate([p, tri, ones], axis=1))


def make_in_maps(x, Wqkv, bqkv, Wo, bo):
    css = _rope_tables()
    cst8 = _cst8()

    Wq = Wqkv[0 * D:1 * D]
    Wk = Wqkv[1 * D:2 * D]
    Wv = Wqkv[2 * D:3 * D]
    bq = bqkv[0 * D:1 * D]
    bk = bqkv[1 * D:2 * D]
    bv = bqkv[2 * D:3 * D]

    in_maps = []
    for c in range(N_CORES):
        b, g = divmod(c, GROUPS)
        hsl = slice(g * HPG * HD, (g + 1) * HPG * HD)
        # de-interleaved row order for q,k heads of this group
        rows = np.arange(g * HPG * HD, (g + 1) * HPG * HD).reshape(HPG, HD)
        rows = rows[:, _DEINT].reshape(-1)

        wq = Wq[rows]                       # (512, D)
        wk = Wk[rows]
        wv = Wv[hsl]                        # natural order
        wqkT = np.concatenate([wq, wk], axis=0).T.astype(np.float32)  # (D, 1024)
        # packed [fb, p, cc*f] in F_ORDER consumption order
        wqpack = wqkT.reshape(NCC, 128, 8, 128)  # (cc, p, fb, f)
        wqpack = wqpack[:, :, F_ORDER, :]
        wqpack = _bf16(
            wqpack.transpose(2, 1, 0, 3).reshape(8, 128, NCC * 128)
        )
        wvT = wv.T.astype(np.float32)            # (D, 512)
        wvpack = _bf16(
            wvT.reshape(NCC, 128, FV).transpose(1, 0, 2).reshape(128, NCC * FV)
        )
        woT = _bf16(Wo[:, hsl].T.astype(np.float32))  # (512, D)

        bqk = np.concatenate([bq[rows], bk[rows]]).astype(np.float32)
        bqk = bqk.reshape(8, 128)[list(F_ORDER)].T  # (128, 8) F_ORDER cols
        bvb = np.broadcast_to(bv[hsl].astype(np.float32), (HD, FV))
        cst32 = np.ascontiguousarray(
            np.concatenate([bqk, bvb], axis=1).astype(np.float32)
        )

        xT = _bf16(np.asarray(x[b]).T)  # (D, T)

        in_maps.append({
            "xT": xT,
            "wqpack": wqpack,
            "wvpack": wvpack,
            "woT": woT,
            "css": css,
            "cst8": cst8,
            "cst32": cst32,
        })
    return in_maps


_NC_CACHE = {}


def _get_nc(loop=1):
    if loop not in _NC_CACHE:
        _NC_CACHE[loop] = build(loop=loop)
    return _NC_CACHE[loop]


def kernel(x, Wqkv, bqkv, Wo, bo):
    x = np.asarray(x)
    Wqkv = np.asarray(Wqkv)
    bqkv = np.asarray(bqkv)
    Wo = np.asarray(Wo)
    bo = np.asarray(bo)

    nc = _get_nc()
    in_maps = make_in_maps(x, Wqkv, bqkv, Wo, bo)
    res = run_bass_kernel_spmd(nc, in_maps, core_ids=list(range(N_CORES)))

    out = np.zeros((B, T, D), dtype=np.float32)
    for c in range(N_CORES):
        b = c // GROUPS
        out[b] += res.results[c]["outp"].astype(np.float32)
    out += bo.astype(np.float32)[None, None, :]
    return out
